# revision 1
# baseline (speedup 1.0000x reference)
"""Trainium2 Bass kernel for DigitConvolutionalModel.

Computation: x[B,784] -> reshape [28,28] -> 3x3 valid conv (single channel)
-> relu -> flatten [676] -> linear to 10 classes.

Strategy (pure data parallel over 8 cores, batch-sharded; per core 4096 rows
in 8 macro-tiles of 512):
  * Host prep: x is cast to bf16 and zero-padded 784 -> 896 columns (7*128).
    The conv is a banded matmul h[o, b] = sum_p Wband[p, o] x^T[p, b] with
    o = 28*oi + oj embedded in 768 slots; Wband bandwidth 59 means each
    128-wide o-chunk needs two 128-row pixel blocks -> 12 conv matmuls per
    macro (all operands bf16, fp32 PSUM accumulation).
  * x^T lands in SBUF directly via ONE DMA-xbar transpose instruction per
    macro: [512, 896] bf16 DRAM -> [128, 7, 512] SBUF (pixel-major chunks).
    No separate load, no tail handling (the pad covers pixels 768..783 that
    o-chunk 5 needs; pad columns have zero weights).
  * ReLU + f32->bf16 cast happens in the PSUM->SBUF drain, round-robined
    across the Activation / DVE / Pool engines; the FC layer is 6
    accumulating matmuls with the embedded fc weight as the stationary
    operand -> out^T [10, b] in PSUM.  Bias is added via tensor_scalar on
    DVE, small PE transposes bring the result back to batch-major, and one
    DMA per macro stores [512, 10] contiguously.
  * After Tile scheduling, a post-pass hoists excess per-instruction
    semaphore waits into standalone EventSemaphore instructions (this
    walrus build only accepts one sync-wait command per instruction).
"""

import sys

for _p in ("/opt/trn_rl_repo",):
    if _p not in sys.path:
        sys.path.insert(0, _p)

import ml_dtypes
import numpy as np

import concourse.bass as bass
import concourse.mybir as mybir
from concourse.bass_utils import run_bass_kernel_spmd
from concourse.tile import TileContext

B = 32768
PIX = 784  # 28*28
PPAD = 896  # 7*128; zero-padded pixel count
EMB = 768  # 6*128; embedded conv-output length (o = 28*oi + oj, max 725)
NCLS = 10
NCORES = 8
BL = B // NCORES  # rows per core
MACRO = 512  # batch rows processed per macro-tile
_BF16 = ml_dtypes.bfloat16


def _host_packs(conv_w, fc_w, fc_b):
    """Pack all constants into three arrays (three DMAs).

    wpack [128, 1596] bf16 = w1 blocks (6x128 cols) | w2 blocks (6x128)
                             | fc chunks (6x10)
    fpack [10, 10] f32     = eye10
    vpack [128, 1] f32     = ones-row selector: 1.0 at row 87, else 0

    The fc bias rides the matmul: embedded slot o=727 (invalid conv output,
    = row 87 of chunk 5) gets fct[727] = fc_b, and the chunk-5 ReLU drain
    adds vpack so ht[87] == 1.
    """
    oi = np.repeat(np.arange(26), 26)
    oj = np.tile(np.arange(26), 26)
    o = 28 * oi + oj  # embedded output index (0..725)
    wband = np.zeros((PPAD, EMB), np.float32)
    for ki in range(3):
        for kj in range(3):
            wband[o + 28 * ki + kj, o] = conv_w[ki, kj]
    cols = [wband[128 * q : 128 * (q + 1), 128 * q : 128 * (q + 1)] for q in range(6)]
    cols += [wband[128 * (q + 1) : 128 * (q + 2), 128 * q : 128 * (q + 1)] for q in range(6)]
    fct = np.zeros((EMB, NCLS), np.float32)
    fct[o, :] = fc_w[:, 26 * oi + oj].T
    fct[727, :] = fc_b
    cols += [fct[128 * q : 128 * (q + 1)] for q in range(6)]
    wpack = np.ascontiguousarray(np.concatenate(cols, axis=1)).astype(_BF16)
    fpack = np.ascontiguousarray(np.eye(NCLS, dtype=np.float32))
    vpack = np.zeros((128, 1), np.float32)
    vpack[727 - 640, 0] = 1.0
    return wpack, fpack, vpack


def _host_x(x):
    """bf16-cast and zero-pad x to [B, 896]."""
    xb = np.zeros((x.shape[0], PPAD), dtype=_BF16)
    xb[:, :PIX] = x.astype(_BF16)
    return xb


def _split_waits(nc, max_waits=1):
    """Hoist excess semaphore waits into standalone EventSemaphore
    instructions (walrus codegen accepts one sync-wait command per
    instruction; the Tile scheduler can attach more)."""
    for f in nc.m.functions:
        for blk in f.blocks:
            new = []
            changed = False
            for inst in blk.instructions:
                si = inst.sync_info
                if si is not None and len(si.on_wait) > max_waits:
                    waits = list(si.on_wait)
                    excess, keep = waits[:-max_waits], waits[-max_waits:]
                    for i, w in enumerate(excess):
                        new.append(
                            mybir.InstEventSemaphore(
                                name=f"{inst.name}-prewait{i}",
                                engine=inst.engine,
                                ins=[],
                                outs=[],
                                sync_info=mybir.SyncInfo(on_wait=[w], on_update=[]),
                            )
                        )
                    inst.sync_info = mybir.SyncInfo(
                        on_wait=keep, on_update=list(si.on_update)
                    )
                    changed = True
                new.append(inst)
            if changed:
                blk.instructions = new


# Tunables.
CFG = {
    "xt_bufs": 6,
    "ht_bufs": 13,
    "hp_bufs": 3,
    "tp_bufs": 2,
    "op_bufs": 2,
    "osb_bufs": 3,
    # 0=ACT 1=DVE per conv chunk (Pool/GPSIMD cannot read PSUM on TRN2 HW);
    # q=5 always runs the DVE tensor_scalar that injects the bias ones-row.
    "drain_order": (0, 1, 0, 1, 0, 1),
    "drain_out": 1,  # engine for the ops PSUM->SBUF copy (0=ACT 1=DVE)
    # Macro-tile batch sizes: small tiles at the start fill the DMA->PE
    # pipeline sooner (first conv waits on only a 128-row transpose), small
    # tiles at the end shorten the drain->fc->bias->store tail.
    "macros": (128, 128, 256, 512, 512, 512, 512, 512, 384, 384, 256),
    "warmup_pe": 3,  # dependency-free PE matmuls at t~0 (p-state ramp)
    "pre_xpose": 2,  # input transposes emitted before the small const loads
}

_DR_RELU = mybir.ActivationFunctionType.Relu


def build_nc(bl=BL, split_waits=True):
    nc = bass.Bass("TRN2")
    xb = nc.dram_tensor("xb", [bl, PPAD], mybir.dt.bfloat16, kind="ExternalInput")
    wpk = nc.dram_tensor("wpack", [128, 1596], mybir.dt.bfloat16, kind="ExternalInput")
    fpk = nc.dram_tensor("fpack", [NCLS, NCLS], mybir.dt.float32, kind="ExternalInput")
    vpk = nc.dram_tensor("vpack", [128, 1], mybir.dt.float32, kind="ExternalInput")
    out = nc.dram_tensor("out", [bl, NCLS], mybir.dt.float32, kind="ExternalOutput")

    macros = list(CFG["macros"])
    if sum(macros) != bl:  # fall back to uniform tiling for other shard sizes
        macros = [MACRO] * (bl // MACRO)

    with TileContext(nc) as tc:
        with (
            tc.tile_pool(name="const", bufs=1) as cp,
            tc.tile_pool(name="xt", bufs=CFG["xt_bufs"]) as xtp,
            tc.tile_pool(name="ht", bufs=CFG["ht_bufs"]) as htp,
            tc.tile_pool(name="osb", bufs=CFG["osb_bufs"]) as osp,
            tc.tile_pool(name="hps", bufs=CFG["hp_bufs"], space="PSUM") as hpp,
            tc.tile_pool(name="tps", bufs=CFG["tp_bufs"], space="PSUM") as tpp,
            tc.tile_pool(name="ops", bufs=CFG["op_bufs"], space="PSUM") as opp,
            tc.tile_pool(name="wup", bufs=1, space="PSUM") as wup,
        ):
            # PE warmup: matmuls on a zeroed tile, dependency-free so they
            # issue immediately and start the p-state ramp clock.
            if CFG["warmup_pe"]:
                zt = cp.tile([128, 128], mybir.dt.bfloat16)
                nc.gpsimd.memset(zt[:, :], 0)
                for w in range(CFG["warmup_pe"]):
                    wtp = wup.tile([128, NCLS], mybir.dt.float32, tag="wu")
                    nc.tensor.matmul(
                        wtp[:, :], zt[:, :], zt[:, 0:NCLS], start=True, stop=True
                    )
                # Preload the Relu activation table (1.3us) while the first
                # input DMAs are in flight, so the first real drain is cheap.
                zo = cp.tile([128, 128], mybir.dt.bfloat16)
                nc.scalar.activation(zo[:, :], zt[:, :], _DR_RELU)

            # vps rides SWDGE (Pool desc-gen) so it lands early without
            # delaying the SP-side wpack -> transpose chain; the first DVE
            # q5-drain needs it.
            vps = cp.tile([128, 1], mybir.dt.float32)
            nc.gpsimd.dma_start(vps[:, :], vpk[:, :])

            wps = cp.tile([128, 1596], mybir.dt.bfloat16)
            nc.sync.dma_start(wps[:, :], wpk[:, :])
            w1s = wps[:, 0:768]
            w2s = wps[:, 768:1536]
            fcs = wps[:, 1536:1596]

            # First input transposes go ahead of the small const loads so the
            # conv pipeline fills as early as possible.
            starts = []
            acc = 0
            for nb in macros:
                starts.append(acc)
                acc += nb
            # xt tiles are allocated at the exact macro size so the xbar
            # transpose's destination AP is fully contiguous (a strided 3D
            # dest was flakily mis-written by the hardware xbar).
            def emit_xpose(pm):
                nb = macros[pm]
                xt = xtp.tile([128, 7, nb], mybir.dt.bfloat16, tag=f"xt{nb}")
                nc.sync.dma_start_transpose(
                    xt[:, :, :], xb[starts[pm] : starts[pm] + nb, :]
                )
                return xt

            pre_xt = {}
            for pm in range(min(CFG["pre_xpose"], len(macros))):
                pre_xt[pm] = emit_xpose(pm)

            fps = cp.tile([NCLS, NCLS], mybir.dt.float32)
            nc.sync.dma_start(fps[:, :], fpk[:, :])
            eyfs = fps[:, :]

            def emit_conv(m, nb, xt):
                """12 conv matmuls + 6 relu drains; returns ht tiles."""
                hts = []
                for q in range(6):
                    hp = hpp.tile([128, MACRO], mybir.dt.float32, tag="hp")
                    nc.tensor.matmul(
                        hp[:, 0:nb], w1s[:, 128 * q : 128 * (q + 1)], xt[:, q, :],
                        start=True, stop=False,
                    )
                    nc.tensor.matmul(
                        hp[:, 0:nb], w2s[:, 128 * q : 128 * (q + 1)], xt[:, q + 1, :],
                        start=False, stop=True,
                    )
                    ht = htp.tile([128, MACRO], mybir.dt.bfloat16, tag="ht")
                    if q == 5:
                        # relu + ones-row injection (row 87 <- 0 + 1.0) so the
                        # fc matmul's fct[727] row adds the bias.
                        nc.vector.tensor_scalar(
                            ht[:, 0:nb], hp[:, 0:nb], 0.0, vps[:, :],
                            op0=mybir.AluOpType.max, op1=mybir.AluOpType.add,
                        )
                    elif CFG["drain_order"][q] == 0:
                        nc.scalar.activation(ht[:, 0:nb], hp[:, 0:nb], _DR_RELU)
                    else:
                        nc.vector.tensor_scalar_max(ht[:, 0:nb], hp[:, 0:nb], 0.0)
                    hts.append(ht)
                return hts

            def emit_fc(nb, hts):
                """6 accumulating fc matmuls + PSUM->SBUF copy of out^T."""
                ops = opp.tile([NCLS, MACRO], mybir.dt.float32, tag="ops")
                for q in range(6):
                    nc.tensor.matmul(
                        ops[:, 0:nb], fcs[:, NCLS * q : NCLS * (q + 1)],
                        hts[q][:, 0:nb], start=(q == 0), stop=(q == 5),
                    )
                ot = osp.tile([NCLS, MACRO], mybir.dt.float32, tag="ot")
                if CFG["drain_out"] == 0:
                    nc.scalar.copy(ot[:, 0:nb], ops[:, 0:nb])
                else:
                    nc.vector.tensor_copy(ot[:, 0:nb], ops[:, 0:nb])
                return ot

            def emit_out(m, nb, ot):
                """PE transposes back to batch-major + one store DMA."""
                r0 = starts[m]
                nbc = nb // 128
                ob4 = osp.tile([128, (MACRO // 128) * NCLS], mybir.dt.float32, tag="ob4")
                for bc in range(nbc):
                    op2 = tpp.tile([128, NCLS], mybir.dt.float32, tag="tp")
                    nc.tensor.transpose(
                        op2[:, :], ot[:, bc * 128 : (bc + 1) * 128], eyfs[:, :]
                    )
                    if bc % 2 == 0:
                        nc.vector.tensor_copy(ob4[:, bc * NCLS : (bc + 1) * NCLS], op2[:, :])
                    else:
                        nc.scalar.copy(ob4[:, bc * NCLS : (bc + 1) * NCLS], op2[:, :])
                nc.sync.dma_start(
                    out[r0 : r0 + nb, :].rearrange("(b p) c -> p b c", p=128),
                    ob4[:, 0 : nbc * NCLS].rearrange("p (b c) -> p b c", c=NCLS),
                )

            # Two-deep software pipeline on the in-order PE queue: iteration m
            # emits conv_m, fc_{m-1}, out_{m-2}, so fc never waits on a drain
            # that was just issued and out-transposes never wait on the ot copy.
            fcq = []   # [(m, nb, hts)]
            outq = []  # [(m, nb, ot)]
            for m, nb in enumerate(macros):
                xt = pre_xt.pop(m) if m in pre_xt else emit_xpose(m)
                hts = emit_conv(m, nb, xt)
                fcq.append((m, nb, hts))
                if len(fcq) > 1:
                    fm, fnb, fhts = fcq.pop(0)
                    outq.append((fm, fnb, emit_fc(fnb, fhts)))
                if len(outq) > 1:
                    om, onb, oot = outq.pop(0)
                    emit_out(om, onb, oot)
            # Flush: emit already-ready out paths first so PE stays busy while
            # the last macro's drains complete, then the last fc + its out.
            while outq:
                om, onb, oot = outq.pop(0)
                emit_out(om, onb, oot)
            while fcq:
                fm, fnb, fhts = fcq.pop(0)
                emit_out(fm, fnb, emit_fc(fnb, fhts))
    if split_waits:
        _split_waits(nc)
    return nc


_CACHED = {}


def _get_nc(bl):
    if bl not in _CACHED:
        _CACHED[bl] = build_nc(bl)
    return _CACHED[bl]


def kernel(x, conv_w, fc_w, fc_b):
    x = np.ascontiguousarray(np.asarray(x, dtype=np.float32))
    conv_w = np.asarray(conv_w, dtype=np.float32)
    fc_w = np.asarray(fc_w, dtype=np.float32)
    fc_b = np.asarray(fc_b, dtype=np.float32)

    wpack, fpack, vpack = _host_packs(conv_w, fc_w, fc_b)
    xbig = _host_x(x)

    nc = _get_nc(BL)
    in_maps = []
    for c in range(NCORES):
        in_maps.append(
            {
                "xb": xbig[c * BL : (c + 1) * BL],
                "wpack": wpack,
                "fpack": fpack,
                "vpack": vpack,
            }
        )
    # The axon-proxied NeuronCores occasionally come up wedged
    # (NRT_EXEC_UNIT_UNRECOVERABLE) on the first execute after idle periods;
    # a retry on a fresh execute reliably recovers.
    last_err = None
    for _attempt in range(3):
        try:
            res = run_bass_kernel_spmd(nc, in_maps, core_ids=list(range(NCORES)))
            break
        except Exception as e:  # noqa: BLE001
            last_err = e
            if "UNRECOVERABLE" not in str(e) and "desynced" not in str(e):
                raise
    else:
        raise last_err
    out = np.concatenate([np.asarray(r["out"]) for r in res.results], axis=0)
    return out


if __name__ == "__main__":
    rng = np.random.default_rng(0)
    xs = rng.standard_normal((B, PIX), dtype=np.float32)
    cw = rng.standard_normal((3, 3), dtype=np.float32)
    fw = (rng.standard_normal((NCLS, 676)) * 0.05).astype(np.float32)
    fb = (rng.standard_normal((NCLS,)) * 0.05).astype(np.float32)
    res = kernel(xs, cw, fw, fb)
    print(res.shape, res.dtype)



# revision 48
# speedup vs baseline: 1.2205x; 1.2205x over previous
"""Trainium2 Bass kernel for DigitConvolutionalModel.

Computation: x[B,784] -> reshape [28,28] -> 3x3 valid conv (single channel)
-> relu -> flatten [676] -> linear to 10 classes.

Strategy (pure data parallel over 8 cores, batch-sharded; per core 4096 rows
in macro-tiles):
  * Host prep: x is cast to bf16 and zero-padded 784 -> 896 columns (7*128).
    The conv is a banded matmul h[o, b] = sum_p Wband[p, o] x^T[p, b] with
    o = 28*oi + oj embedded in 768 slots; Wband bandwidth 59 means each
    128-wide o-chunk needs two 128-row pixel blocks -> 12 conv matmuls per
    macro (all operands bf16, fp32 PSUM accumulation).
  * x^T lands in SBUF via ONE DMA-xbar transpose instruction per macro:
    [nb, 896] bf16 DRAM -> [128, 7, nb] SBUF.  The transposes are split
    across the two HWDGE queues (SP and Activation) so their cost
    (14 ns per 16x128 xbar tile, ~25 us total for the shard) runs on two
    queues in parallel instead of serializing on SP.  The wpack constant
    load is a plain passthrough DMA scheduled between the first ACT
    transpose and the rest: the Tile scheduler's xbar mode-switch
    barriers then stagger the two queues' transpose streams, which is
    REQUIRED for correctness -- schedules where both queues start
    transposing simultaneously at t~200 silently corrupt the transposed
    data on real hardware (CoreSim does not model this).  ALL other DMAs
    stay on the HWDGE class (no Pool/SWDGE DMAs).
  * ReLU + f32->bf16 cast happens in the PSUM->SBUF drain.  Adjacent conv
    chunks accumulate into one two-bank PSUM tile so each drain covers two
    chunks (half the per-instruction access bubbles); drains are split
    between the Activation / DVE engines (Pool cannot read PSUM on TRN2).
  * The FC layer is flipped: the drained h chunk [128o x 128b] is the
    STATIONARY operand and the tiny fc weight chunk [128o, 10] is the
    moving operand, so each fc matmul costs only 10 PE columns (vs 512)
    and the result lands batch-major [128b, 10] in PSUM.  The bias is a
    K=1 closing matmul per batch block: a constant ones-row (stationary)
    against the packed fc_b row (moving).
  * fc PSUM tiles are copied [128, 40] -> SBUF (~170ns each) into one
    shard-wide store buffer; a bulk store ships macros 0..nm-3 off the
    critical path and a tiny store covers the last two macros.
  * After Tile scheduling, a post-pass hoists excess per-instruction
    semaphore waits into standalone EventSemaphore instructions (this
    walrus build only accepts one sync-wait command per instruction).
"""

import sys

for _p in ("/opt/trn_rl_repo",):
    if _p not in sys.path:
        sys.path.insert(0, _p)

import ml_dtypes
import numpy as np

import concourse.bass as bass
import concourse.mybir as mybir
from concourse.bass_utils import run_bass_kernel_spmd
from concourse.tile import TileContext

B = 32768
PIX = 784  # 28*28
PPAD = 896  # 7*128; zero-padded pixel count
EMB = 768  # 6*128; embedded conv-output length (o = 28*oi + oj, max 725)
NCLS = 10
NCORES = 8
BL = B // NCORES  # rows per core
MACRO = 512  # max batch rows per macro-tile (1 PSUM bank per chunk at f32)
_BF16 = ml_dtypes.bfloat16


def _host_packs(conv_w, fc_w, fc_b):
    """Pack all constants into one array (one DMA).

    wpack [128, 1616] bf16 = w1 blocks (6x128 cols) | w2 blocks (6x128)
                             | fc chunks (6x10) | fc_b row (10) | pad
    """
    oi = np.repeat(np.arange(26), 26)
    oj = np.tile(np.arange(26), 26)
    o = 28 * oi + oj  # embedded output index (0..725)
    wband = np.zeros((PPAD, EMB), np.float32)
    for ki in range(3):
        for kj in range(3):
            wband[o + 28 * ki + kj, o] = conv_w[ki, kj]
    cols = [wband[128 * q : 128 * (q + 1), 128 * q : 128 * (q + 1)] for q in range(6)]
    cols += [wband[128 * (q + 1) : 128 * (q + 2), 128 * q : 128 * (q + 1)] for q in range(6)]
    fct = np.zeros((EMB, NCLS), np.float32)
    fct[o, :] = fc_w[:, 26 * oi + oj].T
    cols += [fct[128 * q : 128 * (q + 1)] for q in range(6)]
    fcb = np.zeros((128, NCLS), np.float32)
    fcb[0, :] = fc_b  # bias rides a K=1 matmul against a constant ones-row
    cols += [fcb]
    wpack = np.concatenate(cols, axis=1).astype(_BF16)
    wp = np.zeros((128, 1616), _BF16)
    wp[:, :1606] = wpack
    return np.ascontiguousarray(wp)


def _host_x(x):
    """bf16-cast and zero-pad x to [B, 896]."""
    xb = np.zeros((x.shape[0], PPAD), dtype=_BF16)
    xb[:, :PIX] = x.astype(_BF16)
    return xb


def _split_waits(nc, max_waits=1):
    """Hoist excess semaphore waits into standalone EventSemaphore
    instructions (walrus codegen accepts one sync-wait command per
    instruction; the Tile scheduler can attach more)."""
    for f in nc.m.functions:
        for blk in f.blocks:
            new = []
            changed = False
            for inst in blk.instructions:
                si = inst.sync_info
                if si is not None and len(si.on_wait) > max_waits:
                    waits = list(si.on_wait)
                    excess, keep = waits[:-max_waits], waits[-max_waits:]
                    for i, w in enumerate(excess):
                        new.append(
                            mybir.InstEventSemaphore(
                                name=f"{inst.name}-prewait{i}",
                                engine=inst.engine,
                                ins=[],
                                outs=[],
                                sync_info=mybir.SyncInfo(on_wait=[w], on_update=[]),
                            )
                        )
                    inst.sync_info = mybir.SyncInfo(
                        on_wait=keep, on_update=list(si.on_update)
                    )
                    changed = True
                new.append(inst)
            if changed:
                blk.instructions = new


# Tunables.
CFG = {
    "ht_bufs": 8,  # live double-chunk ht tiles (each [128, 2*nb] bf16)
    "hp_bufs": 3,  # PSUM pair tiles (2 banks each)
    "op_bufs": 2,  # PSUM fc-out tiles (1 bank each)
    # Macro-tile batch sizes; small first tiles fill the DMA->PE pipeline
    # sooner, a smaller last tile shortens the drain->fc->store tail.
    "macros": (128, 128, 256, 512, 512, 512, 512, 512, 512, 256, 256),
    # Which HWDGE queue runs each macro's input transpose: 0=SP 1=ACT.
    # m0 rides SP right after wpack (same queue -> no cross-queue completion
    # chain at the pipeline head); ACT's transposes are front-loaded so its
    # drain capacity is free during the steady state.
    "xpose_eng": (0, 1, 0, 0, 1, 0, 0, 1, 0, 0, 0),
    # Drain engine per (macro, chunk-pair): 3 entries per macro, 0=ACT 1=DVE.
    # Early macros drain on DVE while ACT is still busy with its transposes
    # and the relu-table preload.
    "drain_eng": (
        (1, 1, 1), (1, 1, 1), (1, 1, 1),
        (0, 1, 1), (0, 1, 0), (0, 1, 1),
        (0, 1, 0), (0, 1, 1), (0, 1, 0),
        (0, 1, 1), (0, 1, 0),
    ),
    # Queue for the output stores (0=SP 1=ACT).
    "store_eng": 0,
    # Engine for the per-macro fc PSUM->SBUF copy (0=DVE 1=ACT).
    "ot_eng": (0, 0, 0, 0, 0, 0, 0, 0, 0, 1, 1),
    "warmup_pe": 7,  # dependency-free 512-col PE matmuls at t~0 (p-state ramp)
}

_DR_RELU = mybir.ActivationFunctionType.Relu


def build_nc(bl=BL, split_waits=True):
    nc = bass.Bass("TRN2")
    xb = nc.dram_tensor("xb", [bl, PPAD], mybir.dt.bfloat16, kind="ExternalInput")
    wpk = nc.dram_tensor("wpack", [128, 1616], mybir.dt.bfloat16, kind="ExternalInput")
    out = nc.dram_tensor("out", [bl, NCLS], mybir.dt.float32, kind="ExternalOutput")
    wdbg = None
    if CFG.get("debug_wps"):
        wdbg = nc.dram_tensor(
            "wdbg", [128, 1792], mybir.dt.bfloat16, kind="ExternalOutput"
        )
    xdbg = None
    if CFG.get("debug_xt"):
        xdbg = nc.dram_tensor(
            "xdbg", [128, 7 * bl], mybir.dt.bfloat16, kind="ExternalOutput"
        )

    macros = list(CFG["macros"])
    if sum(macros) != bl:  # fall back to uniform tiling for other shard sizes
        macros = [MACRO] * (bl // MACRO)
    nm = len(macros)
    xpose_eng = (list(CFG["xpose_eng"]) * nm)[:nm]
    drain_eng = (list(CFG["drain_eng"]) * nm)[:nm]

    starts = []
    acc = 0
    for nb in macros:
        starts.append(acc)
        acc += nb

    with TileContext(nc) as tc:
        with (
            tc.tile_pool(name="const", bufs=1) as cp,
            tc.tile_pool(name="xt", bufs=1) as xtp,
            tc.tile_pool(name="ht", bufs=CFG["ht_bufs"]) as htp,
            tc.tile_pool(name="hps", bufs=CFG["hp_bufs"], space="PSUM") as hpp,
            tc.tile_pool(name="ops", bufs=CFG["op_bufs"], space="PSUM") as opp,
        ):
            # PE warmup: matmuls on a zeroed tile, dependency-free so they
            # issue immediately and start the p-state ramp clock while the
            # first input transposes are in flight.  They write into the hp
            # PSUM ring (tag shared with the conv) so no extra PSUM bank is
            # needed; the in-order PE queue keeps the reuse safe.
            zt = cp.tile([128, MACRO], mybir.dt.bfloat16)
            nc.gpsimd.memset(zt[:, :], 0)
            for _w in range(CFG["warmup_pe"]):
                wtp = hpp.tile([128, 2 * MACRO], mybir.dt.float32, tag="hp")
                nc.tensor.matmul(
                    wtp[:, 0:MACRO], zt[:, 0:128], zt[:, :], start=True, stop=True
                )
            # Preload the Relu activation table (1.3us) while the first
            # input DMAs are in flight, so the first real drain is cheap.
            zo = cp.tile([128, 128], mybir.dt.bfloat16)
            nc.scalar.activation(zo[:, :], zt[:, 0:128], _DR_RELU)

            # wpack rides SP ahead of its transposes as a TRANSPOSE-mode
            # DMA; the first conv matmul needs it.
            # One [832, 256] xbar transpose loads wpack (see _host_packs;
            # passthrough DMAs here would split the xbar TRANSPOSE epoch and
            # single-column-tile transposes are mis-written by the HW xbar).
            wpst = cp.tile([128, 1616], mybir.dt.bfloat16, name="wpst")
            nc.sync.dma_start(wpst[:, :], wpk[:, :])
            wps = wpst[:, :]
            if wdbg is not None:
                nc.scalar.dma_start(wdbg[:, :], wps)
            w1s = wps[:, 0:768]
            w2s = wps[:, 768:1536]
            fcs = wps[:, 1536:1596]
            fbs = wps[:, 1596:1606]
            # Constant ones-row for the K=1 bias matmul (memset once).
            ones = cp.tile([1, MACRO], mybir.dt.bfloat16, name="ones")
            nc.gpsimd.memset(ones[:, :], 1.0)

            # One output-store SBUF tile for the whole shard; each macro's
            # fc result is appended by a tiny DVE copy and ONE final store
            # DMA ships it.  A single passthrough store can never split the
            # xbar TRANSPOSE epoch (the scheduler serializes xbar mode
            # switches with full completion barriers).
            ogt = cp.tile([128, (bl // 128) * NCLS], mybir.dt.float32, name="ogt")

            # All input transposes up front: no dependencies, so each HWDGE
            # queue streams its share back-to-back.  xt tiles are allocated
            # at the exact macro size so the xbar transpose's destination AP
            # is fully contiguous (a strided 3D dest was flakily mis-written
            # by the hardware xbar).
            xts = []
            for m, nb in enumerate(macros):
                xt = xtp.tile([128, 7, nb], mybir.dt.bfloat16, tag=f"xt{m}")
                eng = nc.scalar if xpose_eng[m] else nc.sync
                eng.dma_start_transpose(
                    xt[:, :, :], xb[starts[m] : starts[m] + nb, :]
                )
                xts.append(xt)

            def emit_conv(m, nb):
                """12 conv matmuls + 3 double-chunk relu drains; returns the
                3 ht pair tiles (chunks (0,1), (2,3), (4,5) along free)."""
                xt = xts[m]
                hts = []
                for pr in range(3):
                    # Chunk halves sit at fixed offsets 0 / MACRO so each
                    # matmul's accumulation region stays inside one PSUM bank
                    # even for partial macros.  The last (tiny) macro's pairs
                    # use a dedicated bank so its conv never waits on the
                    # previous macro's drains through the ring.
                    hp = hpp.tile([128, 2 * MACRO], mybir.dt.float32, tag="hp")
                    hw = hp.shape[1] // 2
                    for half in range(2):
                        q = 2 * pr + half
                        dst = hp[:, half * hw : half * hw + nb]
                        nc.tensor.matmul(
                            dst, w1s[:, 128 * q : 128 * (q + 1)], xt[:, q, :],
                            start=True, stop=False,
                        )
                        nc.tensor.matmul(
                            dst, w2s[:, 128 * q : 128 * (q + 1)], xt[:, q + 1, :],
                            start=False, stop=True,
                        )
                    ht = htp.tile([128, 2 * MACRO], mybir.dt.bfloat16, tag="ht")
                    hpv = hp[:, :].rearrange("p (h c) -> p h c", h=2)[:, :, 0:nb]  # noqa: E501
                    htv = ht[:, :].rearrange("p (h c) -> p h c", h=2)[:, :, 0:nb]
                    if drain_eng[m][pr] == 0:
                        nc.scalar.activation(htv, hpv, _DR_RELU)
                    else:
                        nc.vector.tensor_scalar_max(htv, hpv, 0.0)
                    hts.append(ht)
                return hts

            def emit_fc(m, nb, hts):
                """Flipped fc: per 128-batch block, 6 accumulating matmuls
                with the drained h chunk as the stationary operand and the
                [128, 10] fc weight chunk as the moving operand -> out
                [128b, 10] batch-major in PSUM; then a ~170ns DVE copy into
                the grouped SBUF store tile."""
                nbc = nb // 128
                ops = opp.tile([128, (MACRO // 128) * NCLS], mybir.dt.float32, tag="ops")
                for bc in range(nbc):
                    for q in range(6):
                        c0 = (q % 2) * MACRO + bc * 128
                        nc.tensor.matmul(
                            ops[:, NCLS * bc : NCLS * (bc + 1)],
                            hts[q // 2][:, c0 : c0 + 128],
                            fcs[:, NCLS * q : NCLS * (q + 1)],
                            start=(q == 0), stop=False,
                        )
                    # K=1 closing matmul adds the bias: ones^T @ fc_b row.
                    nc.tensor.matmul(
                        ops[:, NCLS * bc : NCLS * (bc + 1)],
                        ones[:, bc * 128 : bc * 128 + 128],
                        fbs[0:1, :],
                        start=False, stop=True,
                    )
                return ops

            ot_eng = (list(CFG["ot_eng"]) * nm)[:nm]

            def emit_out(m, nb, ops):
                nbc = nb // 128
                c0 = (starts[m] // 128) * NCLS
                w = nbc * NCLS
                if ot_eng[m]:
                    nc.scalar.copy(ogt[:, c0 : c0 + w], ops[:, 0:w])
                else:
                    nc.vector.tensor_copy(ogt[:, c0 : c0 + w], ops[:, 0:w])
                seng = nc.scalar if CFG["store_eng"] else nc.sync
                # The bulk store ships everything but the last macro as soon
                # as its ot copy lands; only a tiny store remains on the
                # critical tail.  Both run after the xbar TRANSPOSE epoch.
                if m == nm - 3:
                    r = starts[nm - 2]
                    seng.dma_start(
                        out[0:r, :].rearrange("(b p) c -> p b c", p=128),
                        ogt[:, 0 : (r // 128) * NCLS].rearrange(
                            "p (b c) -> p b c", c=NCLS
                        ),
                    )
                elif m == nm - 1:
                    r = starts[nm - 2]
                    seng = nc.sync if CFG["store_eng"] else nc.scalar
                    seng.dma_start(
                        out[r:bl, :].rearrange("(b p) c -> p b c", p=128),
                        ogt[:, (r // 128) * NCLS :].rearrange(
                            "p (b c) -> p b c", c=NCLS
                        ),
                    )

            # Two-deep software pipeline on the in-order PE queue: iteration m
            # emits conv_m then fc_{m-1}, so the fc matmuls never wait on a
            # drain that was just issued.
            fcq = []  # [(m, nb, hts)]
            for m, nb in enumerate(macros):
                hts = emit_conv(m, nb)
                fcq.append((m, nb, hts))
                if len(fcq) > 1:
                    fm, fnb, fhts = fcq.pop(0)
                    emit_out(fm, fnb, emit_fc(fm, fnb, fhts))
            while fcq:
                fm, fnb, fhts = fcq.pop(0)
                emit_out(fm, fnb, emit_fc(fm, fnb, fhts))
            if xdbg is not None:
                for m, nb in enumerate(macros):
                    nc.scalar.dma_start(
                        xdbg[:, 7 * starts[m] : 7 * (starts[m] + nb)],
                        xts[m].rearrange("p h r -> p (h r)"),
                    )
    if split_waits:
        _split_waits(nc)
    return nc


_CACHED = {}


def _get_nc(bl):
    if bl not in _CACHED:
        _CACHED[bl] = build_nc(bl)
    return _CACHED[bl]


def kernel(x, conv_w, fc_w, fc_b):
    x = np.ascontiguousarray(np.asarray(x, dtype=np.float32))
    conv_w = np.asarray(conv_w, dtype=np.float32)
    fc_w = np.asarray(fc_w, dtype=np.float32)
    fc_b = np.asarray(fc_b, dtype=np.float32)

    wpack = _host_packs(conv_w, fc_w, fc_b)
    xbig = _host_x(x)

    nc = _get_nc(BL)
    in_maps = []
    for c in range(NCORES):
        in_maps.append(
            {
                "xb": xbig[c * BL : (c + 1) * BL],
                "wpack": wpack,
            }
        )
    # The axon-proxied NeuronCores occasionally come up wedged
    # (NRT_EXEC_UNIT_UNRECOVERABLE) on the first execute after idle periods;
    # a retry on a fresh execute reliably recovers.
    last_err = None
    for _attempt in range(3):
        try:
            res = run_bass_kernel_spmd(nc, in_maps, core_ids=list(range(NCORES)))
            break
        except Exception as e:  # noqa: BLE001
            last_err = e
            if "UNRECOVERABLE" not in str(e) and "desynced" not in str(e):
                raise
    else:
        raise last_err
    out = np.concatenate([np.asarray(r["out"]) for r in res.results], axis=0)
    return out


if __name__ == "__main__":
    rng = np.random.default_rng(0)
    xs = rng.standard_normal((B, PIX), dtype=np.float32)
    cw = rng.standard_normal((3, 3), dtype=np.float32)
    fw = (rng.standard_normal((NCLS, 676)) * 0.05).astype(np.float32)
    fb = (rng.standard_normal((NCLS,)) * 0.05).astype(np.float32)
    res = kernel(xs, cw, fw, fb)
    print(res.shape, res.dtype)


# revision 61
# speedup vs baseline: 1.3111x; 1.0742x over previous
"""Trainium2 Bass kernel for DigitConvolutionalModel.

Computation: x[B,784] -> reshape [28,28] -> 3x3 valid conv (single channel)
-> relu -> flatten [676] -> linear to 10 classes.

Strategy (pure data parallel over 8 cores, batch-sharded; per core 4096 rows
in macro-tiles):
  * Host prep: x is cast to bf16 and zero-padded 784 -> 896 columns (7*128).
    The conv is a banded matmul h[o, b] = sum_p Wband[p, o] x^T[p, b] with
    o = 28*oi + oj embedded in 768 slots; Wband bandwidth 59 means each
    128-wide o-chunk needs two 128-row pixel blocks -> 12 conv matmuls per
    macro (all operands bf16, fp32 PSUM accumulation).
  * x^T lands in SBUF via ONE DMA-xbar transpose instruction per macro:
    [nb, 896] bf16 DRAM -> [128, 7, nb] SBUF.  The transposes are split
    across the two HWDGE queues (SP and Activation) so their cost
    (14 ns per 16x128 xbar tile, ~25 us total for the shard) runs on two
    queues in parallel instead of serializing on SP.  The wpack constant
    load is a plain passthrough DMA scheduled between the first ACT
    transpose and the rest: the Tile scheduler's xbar mode-switch
    barriers then stagger the two queues' transpose streams, which is
    REQUIRED for correctness -- schedules where both queues start
    transposing simultaneously at t~200 silently corrupt the transposed
    data on real hardware (CoreSim does not model this).  ALL other DMAs
    stay on the HWDGE class (no Pool/SWDGE DMAs).
  * ReLU + f32->bf16 cast happens in the PSUM->SBUF drain.  Adjacent conv
    chunks accumulate into one two-bank PSUM tile so each drain covers two
    chunks (half the per-instruction access bubbles); drains are split
    between the Activation / DVE engines (Pool cannot read PSUM on TRN2).
  * The FC layer is flipped: the drained h chunk [128o x 128b] is the
    STATIONARY operand and the tiny fc weight chunk [128o, 10] is the
    moving operand, so each fc matmul costs only 10 PE columns (vs 512)
    and the result lands batch-major [128b, 10] in PSUM.  The bias is a
    K=1 closing matmul per batch block: a constant ones-row (stationary)
    against the packed fc_b row (moving).
  * fc PSUM tiles are copied [128, 40] -> SBUF (~170ns each) into one
    shard-wide store buffer; a bulk store ships macros 0..nm-3 off the
    critical path and a tiny store covers the last two macros.
  * After Tile scheduling, a post-pass hoists excess per-instruction
    semaphore waits into standalone EventSemaphore instructions (this
    walrus build only accepts one sync-wait command per instruction).
"""

import sys

for _p in ("/opt/trn_rl_repo",):
    if _p not in sys.path:
        sys.path.insert(0, _p)

import ml_dtypes
import numpy as np

import concourse.bass as bass
import concourse.mybir as mybir
from concourse.bass_utils import run_bass_kernel_spmd
from concourse.tile import TileContext

B = 32768
PIX = 784  # 28*28
PPAD = 896  # 7*128; zero-padded pixel count
EMB = 768  # 6*128; embedded conv-output length (o = 28*oi + oj, max 725)
NCLS = 10
NCORES = 8
BL = B // NCORES  # rows per core
MACRO = 512  # max batch rows per macro-tile (1 PSUM bank per chunk at f32)
_BF16 = ml_dtypes.bfloat16


def _host_packs(conv_w, fc_w, fc_b):
    """Pack all constants into one array (one DMA).

    wpack [128, 1616] bf16 = w1 blocks (6x128 cols) | w2 blocks (6x128)
                             | fc chunks (6x10) | fc_b row (10) | pad
    """
    oi = np.repeat(np.arange(26), 26)
    oj = np.tile(np.arange(26), 26)
    o = 28 * oi + oj  # embedded output index (0..725)
    wband = np.zeros((PPAD, EMB), np.float32)
    for ki in range(3):
        for kj in range(3):
            wband[o + 28 * ki + kj, o] = conv_w[ki, kj]
    cols = [wband[128 * q : 128 * (q + 1), 128 * q : 128 * (q + 1)] for q in range(6)]
    cols += [wband[128 * (q + 1) : 128 * (q + 2), 128 * q : 128 * (q + 1)] for q in range(6)]
    fct = np.zeros((EMB, NCLS), np.float32)
    fct[o, :] = fc_w[:, 26 * oi + oj].T
    cols += [fct[128 * q : 128 * (q + 1)] for q in range(6)]
    fcb = np.zeros((128, NCLS), np.float32)
    fcb[0, :] = fc_b  # bias rides a K=1 matmul against a constant ones-row
    cols += [fcb]
    wpack = np.concatenate(cols, axis=1).astype(_BF16)
    wp = np.zeros((128, 1616), _BF16)
    wp[:, :1606] = wpack
    return np.ascontiguousarray(wp)


def _host_x(x):
    """bf16-cast and zero-pad x to [B, 896]."""
    xb = np.zeros((x.shape[0], PPAD), dtype=_BF16)
    xb[:, :PIX] = x.astype(_BF16)
    return xb


def _split_waits(nc, max_waits=1):
    """Hoist excess semaphore waits into standalone EventSemaphore
    instructions (walrus codegen accepts one sync-wait command per
    instruction; the Tile scheduler can attach more)."""
    for f in nc.m.functions:
        for blk in f.blocks:
            new = []
            changed = False
            for inst in blk.instructions:
                si = inst.sync_info
                if si is not None and len(si.on_wait) > max_waits:
                    waits = list(si.on_wait)
                    excess, keep = waits[:-max_waits], waits[-max_waits:]
                    for i, w in enumerate(excess):
                        new.append(
                            mybir.InstEventSemaphore(
                                name=f"{inst.name}-prewait{i}",
                                engine=inst.engine,
                                ins=[],
                                outs=[],
                                sync_info=mybir.SyncInfo(on_wait=[w], on_update=[]),
                            )
                        )
                    inst.sync_info = mybir.SyncInfo(
                        on_wait=keep, on_update=list(si.on_update)
                    )
                    changed = True
                new.append(inst)
            if changed:
                blk.instructions = new


# Tunables.
CFG = {
    "ht_bufs": 8,  # live double-chunk ht tiles (each [128, 2*nb] bf16)
    "hp_bufs": 3,  # PSUM pair tiles (2 banks each)
    "op_bufs": 2,  # PSUM fc-out tiles (1 bank each)
    # Macro-tile batch sizes; small first tiles fill the DMA->PE pipeline
    # sooner, a smaller last tile shortens the drain->fc->store tail.
    "macros": (128, 128, 256, 512, 512, 512, 512, 512, 512, 256, 256),
    # Which HWDGE queue runs each macro's input transpose: 0=SP 1=ACT.
    # m0 rides SP right after wpack (same queue -> no cross-queue completion
    # chain at the pipeline head); ACT's transposes are front-loaded so its
    # drain capacity is free during the steady state.
    "xpose_eng": (0, 1, 0, 0, 1, 0, 0, 1, 0, 0, 0),
    # Drain engine per (macro, chunk-pair): 3 entries per macro, 0=ACT 1=DVE.
    # Early macros drain on DVE while ACT is still busy with its transposes
    # and the relu-table preload.
    "drain_eng": (
        (1, 1, 1), (1, 1, 1), (1, 1, 1),
        (0, 1, 1), (0, 1, 0), (0, 1, 1),
        (0, 1, 0), (0, 1, 1), (0, 1, 0),
        (0, 1, 1), (0, 1, 0),
    ),
    # Queue for the output stores (0=SP 1=ACT).
    "store_eng": 0,
    # Engine for the per-macro fc PSUM->SBUF copy (0=DVE 1=ACT).
    "ot_eng": (0, 0, 0, 0, 0, 0, 0, 0, 0, 1, 1),
    "warmup_pe": 7,  # dependency-free 512-col PE matmuls at t~0 (p-state ramp)
}

_DR_RELU = mybir.ActivationFunctionType.Relu


def build_nc(bl=BL, split_waits=True):
    nc = bass.Bass("TRN2")
    xb = nc.dram_tensor("xb", [bl, PPAD], mybir.dt.bfloat16, kind="ExternalInput")
    wpk = nc.dram_tensor("wpack", [128, 1616], mybir.dt.bfloat16, kind="ExternalInput")
    out = nc.dram_tensor("out", [bl, NCLS], mybir.dt.float32, kind="ExternalOutput")
    wdbg = None
    if CFG.get("debug_wps"):
        wdbg = nc.dram_tensor(
            "wdbg", [128, 1792], mybir.dt.bfloat16, kind="ExternalOutput"
        )
    xdbg = None
    if CFG.get("debug_xt"):
        xdbg = nc.dram_tensor(
            "xdbg", [128, 7 * bl], mybir.dt.bfloat16, kind="ExternalOutput"
        )

    macros = list(CFG["macros"])
    if sum(macros) != bl:  # fall back to uniform tiling for other shard sizes
        macros = [MACRO] * (bl // MACRO)
    nm = len(macros)
    xpose_eng = (list(CFG["xpose_eng"]) * nm)[:nm]
    drain_eng = (list(CFG["drain_eng"]) * nm)[:nm]

    starts = []
    acc = 0
    for nb in macros:
        starts.append(acc)
        acc += nb

    with TileContext(nc) as tc:
        with (
            tc.tile_pool(name="const", bufs=1) as cp,
            tc.tile_pool(name="xt", bufs=1) as xtp,
            tc.tile_pool(name="ht", bufs=CFG["ht_bufs"]) as htp,
            tc.tile_pool(name="hps", bufs=CFG["hp_bufs"], space="PSUM") as hpp,
            tc.tile_pool(name="ops", bufs=CFG["op_bufs"], space="PSUM") as opp,
        ):
            # PE warmup: matmuls on a zeroed tile, dependency-free so they
            # issue immediately and start the p-state ramp clock while the
            # first input transposes are in flight.  They write into the hp
            # PSUM ring (tag shared with the conv) so no extra PSUM bank is
            # needed; the in-order PE queue keeps the reuse safe.
            zt = cp.tile([128, MACRO], mybir.dt.bfloat16)
            nc.gpsimd.memset(zt[:, :], 0)
            for _w in range(CFG["warmup_pe"]):
                wtp = hpp.tile([128, 2 * MACRO], mybir.dt.float32, tag="hp")
                nc.tensor.matmul(
                    wtp[:, 0:MACRO], zt[:, 0:128], zt[:, :], start=True, stop=True
                )
            # Preload the Relu activation table (1.3us) while the first
            # input DMAs are in flight, so the first real drain is cheap.
            # Dependency-free explicit table load so ACT's stream advances.
            nc.scalar.add_instruction(
                mybir.InstLoadActFuncSet(
                    name=nc.get_next_instruction_name(),
                    engine=mybir.EngineType.Activation,
                    ins=[],
                    outs=[],
                    act_func_set_id=1,
                )
            )
            zo = cp.tile([128, 128], mybir.dt.bfloat16)
            nc.scalar.activation(zo[:, :], zt[:, 0:128], _DR_RELU)

            # wpack rides SP ahead of its transposes as a TRANSPOSE-mode
            # DMA; the first conv matmul needs it.
            # One [832, 256] xbar transpose loads wpack (see _host_packs;
            # passthrough DMAs here would split the xbar TRANSPOSE epoch and
            # single-column-tile transposes are mis-written by the HW xbar).
            wpst = cp.tile([128, 1616], mybir.dt.bfloat16, name="wpst")
            nc.sync.dma_start(wpst[:, :], wpk[:, :])
            wps = wpst[:, :]
            if wdbg is not None:
                nc.scalar.dma_start(wdbg[:, :], wps)
            w1s = wps[:, 0:768]
            w2s = wps[:, 768:1536]
            fcs = wps[:, 1536:1596]
            fbs = wps[:, 1596:1606]
            # Constant ones-row for the K=1 bias matmul (memset once).
            ones = cp.tile([1, MACRO], mybir.dt.bfloat16, name="ones")
            nc.gpsimd.memset(ones[:, :], 1.0)

            # One output-store SBUF tile for the whole shard; each macro's
            # fc result is appended by a tiny DVE copy and ONE final store
            # DMA ships it.  A single passthrough store can never split the
            # xbar TRANSPOSE epoch (the scheduler serializes xbar mode
            # switches with full completion barriers).
            ogt = cp.tile([128, (bl // 128) * NCLS], mybir.dt.float32, name="ogt")

            # All input transposes up front: no dependencies, so each HWDGE
            # queue streams its share back-to-back.  xt tiles are allocated
            # at the exact macro size so the xbar transpose's destination AP
            # is fully contiguous (a strided 3D dest was flakily mis-written
            # by the hardware xbar).
            xts = []
            for m, nb in enumerate(macros):
                xt = xtp.tile([128, 7, nb], mybir.dt.bfloat16, tag=f"xt{m}")
                eng = nc.scalar if xpose_eng[m] else nc.sync
                eng.dma_start_transpose(
                    xt[:, :, :], xb[starts[m] : starts[m] + nb, :]
                )
                xts.append(xt)

            def emit_conv(m, nb):
                """12 conv matmuls + 3 double-chunk relu drains; returns the
                3 ht pair tiles (chunks (0,1), (2,3), (4,5) along free)."""
                xt = xts[m]
                hts = []
                for pr in range(3):
                    # Chunk halves sit at fixed offsets 0 / MACRO so each
                    # matmul's accumulation region stays inside one PSUM bank
                    # even for partial macros.  The last (tiny) macro's pairs
                    # use a dedicated bank so its conv never waits on the
                    # previous macro's drains through the ring.
                    hp = hpp.tile([128, 2 * MACRO], mybir.dt.float32, tag="hp")
                    hw = hp.shape[1] // 2
                    for half in range(2):
                        q = 2 * pr + half
                        dst = hp[:, half * hw : half * hw + nb]
                        nc.tensor.matmul(
                            dst, w1s[:, 128 * q : 128 * (q + 1)], xt[:, q, :],
                            start=True, stop=False,
                        )
                        nc.tensor.matmul(
                            dst, w2s[:, 128 * q : 128 * (q + 1)], xt[:, q + 1, :],
                            start=False, stop=True,
                        )
                    ht = htp.tile([128, 2 * MACRO], mybir.dt.bfloat16, tag="ht")
                    hpv = hp[:, :].rearrange("p (h c) -> p h c", h=2)[:, :, 0:nb]  # noqa: E501
                    htv = ht[:, :].rearrange("p (h c) -> p h c", h=2)[:, :, 0:nb]
                    if drain_eng[m][pr] == 0:
                        nc.scalar.activation(htv, hpv, _DR_RELU)
                    else:
                        nc.vector.tensor_scalar_max(htv, hpv, 0.0)
                    hts.append(ht)
                return hts

            def emit_fc(m, nb, hts):
                """Flipped fc: per 128-batch block, 6 accumulating matmuls
                with the drained h chunk as the stationary operand and the
                [128, 10] fc weight chunk as the moving operand -> out
                [128b, 10] batch-major in PSUM; then a ~170ns DVE copy into
                the grouped SBUF store tile."""
                nbc = nb // 128
                ops = opp.tile([128, (MACRO // 128) * NCLS], mybir.dt.float32, tag="ops")
                for bc in range(nbc):
                    for q in range(6):
                        c0 = (q % 2) * MACRO + bc * 128
                        nc.tensor.matmul(
                            ops[:, NCLS * bc : NCLS * (bc + 1)],
                            hts[q // 2][:, c0 : c0 + 128],
                            fcs[:, NCLS * q : NCLS * (q + 1)],
                            start=(q == 0), stop=False,
                        )
                    # K=1 closing matmul adds the bias: ones^T @ fc_b row.
                    nc.tensor.matmul(
                        ops[:, NCLS * bc : NCLS * (bc + 1)],
                        ones[:, bc * 128 : bc * 128 + 128],
                        fbs[0:1, :],
                        start=False, stop=True,
                    )
                return ops

            ot_eng = (list(CFG["ot_eng"]) * nm)[:nm]

            def emit_out(m, nb, ops):
                nbc = nb // 128
                c0 = (starts[m] // 128) * NCLS
                w = nbc * NCLS
                if ot_eng[m]:
                    nc.scalar.copy(ogt[:, c0 : c0 + w], ops[:, 0:w])
                else:
                    nc.vector.tensor_copy(ogt[:, c0 : c0 + w], ops[:, 0:w])
                seng = nc.scalar if CFG["store_eng"] else nc.sync
                # The bulk store ships everything but the last macro as soon
                # as its ot copy lands; only a tiny store remains on the
                # critical tail.  Both run after the xbar TRANSPOSE epoch.
                if m == nm - 3:
                    r = starts[nm - 2]
                    seng.dma_start(
                        out[0:r, :].rearrange("(b p) c -> p b c", p=128),
                        ogt[:, 0 : (r // 128) * NCLS].rearrange(
                            "p (b c) -> p b c", c=NCLS
                        ),
                    )
                elif m == nm - 1:
                    r = starts[nm - 2]
                    seng = nc.sync if CFG["store_eng"] else nc.scalar
                    seng.dma_start(
                        out[r:bl, :].rearrange("(b p) c -> p b c", p=128),
                        ogt[:, (r // 128) * NCLS :].rearrange(
                            "p (b c) -> p b c", c=NCLS
                        ),
                    )

            # Two-deep software pipeline on the in-order PE queue: iteration m
            # emits conv_m then fc_{m-1}, so the fc matmuls never wait on a
            # drain that was just issued.
            fcq = []  # [(m, nb, hts)]
            for m, nb in enumerate(macros):
                hts = emit_conv(m, nb)
                fcq.append((m, nb, hts))
                if len(fcq) > 1:
                    fm, fnb, fhts = fcq.pop(0)
                    emit_out(fm, fnb, emit_fc(fm, fnb, fhts))
            while fcq:
                fm, fnb, fhts = fcq.pop(0)
                emit_out(fm, fnb, emit_fc(fm, fnb, fhts))
            if xdbg is not None:
                for m, nb in enumerate(macros):
                    nc.scalar.dma_start(
                        xdbg[:, 7 * starts[m] : 7 * (starts[m] + nb)],
                        xts[m].rearrange("p h r -> p (h r)"),
                    )
    if split_waits:
        _split_waits(nc)
    return nc


_CACHED = {}


def _get_nc(bl):
    if bl not in _CACHED:
        _CACHED[bl] = build_nc(bl)
    return _CACHED[bl]


def kernel(x, conv_w, fc_w, fc_b):
    x = np.ascontiguousarray(np.asarray(x, dtype=np.float32))
    conv_w = np.asarray(conv_w, dtype=np.float32)
    fc_w = np.asarray(fc_w, dtype=np.float32)
    fc_b = np.asarray(fc_b, dtype=np.float32)

    wpack = _host_packs(conv_w, fc_w, fc_b)
    xbig = _host_x(x)

    nc = _get_nc(BL)
    in_maps = []
    for c in range(NCORES):
        in_maps.append(
            {
                "xb": xbig[c * BL : (c + 1) * BL],
                "wpack": wpack,
            }
        )
    # The axon-proxied NeuronCores occasionally come up wedged
    # (NRT_EXEC_UNIT_UNRECOVERABLE) on the first execute after idle periods;
    # a retry on a fresh execute reliably recovers.
    last_err = None
    for _attempt in range(3):
        try:
            res = run_bass_kernel_spmd(nc, in_maps, core_ids=list(range(NCORES)))
            break
        except Exception as e:  # noqa: BLE001
            last_err = e
            if "UNRECOVERABLE" not in str(e) and "desynced" not in str(e):
                raise
    else:
        raise last_err
    out = np.concatenate([np.asarray(r["out"]) for r in res.results], axis=0)
    return out


if __name__ == "__main__":
    rng = np.random.default_rng(0)
    xs = rng.standard_normal((B, PIX), dtype=np.float32)
    cw = rng.standard_normal((3, 3), dtype=np.float32)
    fw = (rng.standard_normal((NCLS, 676)) * 0.05).astype(np.float32)
    fb = (rng.standard_normal((NCLS,)) * 0.05).astype(np.float32)
    res = kernel(xs, cw, fw, fb)
    print(res.shape, res.dtype)
